# revision 23
# baseline (speedup 1.0000x reference)
"""AuthPct metric kernel for 8 Trainium2 NeuronCores.

Per core c: rows i = real shard c (1536, moving operand), columns j in
128-wide tiles.  The Gram term 16*f_j.r_i is computed by ONE fp8-e4m3
DoubleRow matmul per 512-i PSUM bank (K=256 packed [128,2,*]); real
tiles add the hi/lo-bf16 norm aug matmul (-8|r_i|^2 - 8|r_j|^2) like the
original baseline, so real PSUM holds complete -8*d^2.

gen side (96 tiles): a single custom DVE op (PACK_BIAS, registered into
dve_ops.OPS at runtime) scans each 3-bank [128,1536] f32 PSUM tile:

    q = round(Src0); P = q*2048 + Src1; accum_out = max_i(P)
    (Src1 payload = round(-8|r_i|^2)*2048 + i)

one 1x pass yields the quantized column max (d^2 to 1/16) AND its argmin
index in the low 11 bits -- replacing the old max + max_index two-pass.

real side (symmetric, shards c..c+4 rotated): every real tile is
evacuated per-bank by ACT to an fp16 SBUF copy; the j-side minima for
ALL 60 tiles (coverage sources t-4..t) use fp16 tensor_mask_reduce at
DVE 2x (the m=0 self tile masks out its diagonal via the wrap-around
start=d+1,end=d window); the i-side minima use Pool partition_all_reduce
on only the m=1..3 copies (sources t+1..t+3) -- 36 PARs instead of the
baseline's 48, since free-side coverage grew to 5 shards.

Host combine: decode q=floor(P/2048), idx=P mod 2048 for gen; real/PAR
values are plain fp16-rounded maxima of -8*d^2; min-combine across
cores, d2 = realNN[argmin], sigmoid, mean.

Measured (HW trace, 306us span): DVE 267us busy (96 packs ~1.86us, 48
tensor_reduce ~1.74us, 12 custom tmr), Pool 36 PAR x 5.4us = 195us, PE
635 matmuls 193us, ACT 180 bank copies 123us, DMA 166us -- DVE-bound.
"""

import os
import numpy as np

NO_DR = bool(int(os.environ.get("V5_NO_DR", "0")))
NO_TMR = bool(int(os.environ.get("V5_NO_TMR", "0")))
NO_PAR = bool(int(os.environ.get("V5_NO_PAR", "0")))

N = 12288
D = 256
NCORES = 8
SHARD = N // NCORES          # 1536 rows per core
JTILE = 128                  # j columns per tile (PSUM partitions)
NJT = N // JTILE             # 96 gen j-tiles
RJT = 60                     # real j-tiles: shards c..c+4 (rotated)
NPAR = 36                    # real j-tiles with PAR harvest (m=1..3)
NT = 512                     # i elements per matmul (PSUM bank)
NIT = SHARD // NT            # 3 i-tiles

M_ROUND = 12582912.0         # 1.5*2^23
PSCALE = 2048.0
FMIN = -3.4028234663852886e38

_cached_nc = None
_pack_ops = None


def _register_pack_ops():
    """Register the PACK_BIAS custom DVE op (idempotent)."""
    global _pack_ops
    if _pack_ops is not None:
        return _pack_ops
    import concourse.dve_ops as dve_ops
    from concourse.dve_spec import (
        Spec, Src0, Src1, C0, C1, C2, MaxNeg, maxx, select, lower,
    )
    from concourse.dve_uop import DveOpSpec
    from concourse.dve_ops import has_src1

    if "PACK_BIAS_ANT" in dve_ops._SUB_OPCODE_FOR_NAME:
        by_name = {op.name: op for op in dve_ops.OPS}
        _pack_ops = (by_name["PACK_BIAS_ANT"],)
        return _pack_ops

    def ref_bias(in0, in1, c0, c1, c2):
        x = np.asarray(in0, np.float32)
        c0 = np.float32(c0) if not isinstance(c0, np.ndarray) else c0.astype(np.float32)
        q = (np.float32(x + c0) - c0).astype(np.float32)
        P = (q * np.float32(c2) + np.asarray(in1, np.float32)).astype(np.float32)
        return P, P.max(axis=-1)

    q = (Src0 + C0) - C0
    P = q * C2 + Src1
    spec_bias = Spec(body=P, accum=maxx, reference=ref_bias)

    ops = []
    for name, spec in (("PACK_BIAS_ANT", spec_bias),):
        row = dve_ops._CUSTOM_DVE_ROW_BASE + len(dve_ops.OPS)
        dve_ops._SUB_OPCODE_FOR_NAME[name] = row
        shas = {}
        for ver in ("v3", "v4"):
            tmp = DveOpSpec(name=name, opcode=row, uops=lower(spec, ver=ver),
                            rd1_en=has_src1(spec))
            shas[ver] = tmp.sha(ver)
        op = dve_ops.DveOp(name, spec, subdim=False, uops_sha=shas)
        dve_ops.OPS.append(op)
        dve_ops.CUSTOM_DVE_SPECS[name] = spec
        ops.append(op)
    assert max(dve_ops._SUB_OPCODE_FOR_NAME.values()) < 0x20
    _pack_ops = tuple(ops)
    return _pack_ops


def _build_nc():
    import concourse.bass_isa as bass_isa
    import concourse.mybir as mybir
    from concourse import bacc
    from concourse.tile import TileContext

    (PACK_BIAS,) = _register_pack_ops()

    f32 = mybir.dt.float32
    fp16 = mybir.dt.float16
    bf16 = mybir.dt.bfloat16
    fp8 = mybir.dt.float8e4

    nc = bacc.Bacc("TRN2", target_bir_lowering=False, debug=False,
                   num_devices=NCORES)

    RW = RJT * JTILE
    # fp8 lhs/rhs packed for DoubleRow: [p, s, x] with k = s*128+p
    gdt = bf16 if NO_DR else fp8
    colr = nc.dram_tensor("colr", [128, 2, RW], gdt, kind="ExternalInput")
    colg = nc.dram_tensor("colg", [128, 2, N], gdt, kind="ExternalInput")
    rhs = nc.dram_tensor("rhs", [128, 2, SHARD], gdt, kind="ExternalInput")
    auglr = nc.dram_tensor("auglr", [JTILE, RW], bf16, kind="ExternalInput")
    augr = nc.dram_tensor("augr", [128, SHARD], bf16, kind="ExternalInput")
    payf = nc.dram_tensor("payf", [128, SHARD], f32, kind="ExternalInput")
    mstart = nc.dram_tensor("mstart", [128, 12], f32, kind="ExternalInput")
    mend = nc.dram_tensor("mend", [128, 12], f32, kind="ExternalInput")

    o_gen = nc.dram_tensor("o_gen", [128, NJT], f32, kind="ExternalOutput")
    o_real = nc.dram_tensor("o_real", [128, RJT], f32, kind="ExternalOutput")
    o_par = nc.dram_tensor("o_par", [NPAR, SHARD], f32, kind="ExternalOutput")

    with TileContext(nc) as tc:
        with (
            tc.tile_pool(name="const", bufs=1) as constp,
            tc.tile_pool(name="lhs", bufs=10) as lhsp,
            tc.tile_pool(name="scr", bufs=4) as scrp,
            tc.tile_pool(name="pari", bufs=10) as parip,
            tc.tile_pool(name="paro", bufs=6) as parop,
            tc.tile_pool(name="outb", bufs=1) as outp,
            tc.tile_pool(name="psg", bufs=2, space="PSUM") as psgp,
            tc.tile_pool(name="psr", bufs=2, space="PSUM") as psrp,
        ):
            # Payload first: the very first gen pack blocks on it.
            payf_sb = constp.tile([128, SHARD], f32)
            nc.sync.dma_start(out=payf_sb[:, :], in_=payf[:, :])
            # Resident moving operand (fp8 DR layout) + aug rows.
            rhs_sb = constp.tile([128, 2 * SHARD], gdt)
            rhs_v = rhs_sb[:, :].rearrange("p (s n) -> p s n", s=2)
            nc.sync.dma_start(out=rhs_v[:, :, 0:NT], in_=rhs[:, :, 0:NT])
            augr_sb = constp.tile([128, SHARD], bf16)
            nc.sync.dma_start(out=augr_sb[:, 0:NT], in_=augr[:, 0:NT])
            mst_sb = constp.tile([128, 12], f32)
            nc.sync.dma_start(out=mst_sb[:, :], in_=mstart[:, :])
            men_sb = constp.tile([128, 12], f32)
            nc.sync.dma_start(out=men_sb[:, :], in_=mend[:, :])
            fullend_sb = constp.tile([128, 1], f32)
            nc.vector.memset(fullend_sb[:, :], float(SHARD))

            geno = outp.tile([128, NJT], f32)
            realo = outp.tile([128, RJT], f32)

            def emit_real(rjt, lhs_r_v, auglr_t):
                pin = parip.tile([128, SHARD], fp16, tag="pin")
                for it in range(NIT):
                    io = it * NT
                    ps_r = psrp.tile([128, NT], f32, tag="psr")
                    if NO_DR:
                        nc.tensor.matmul(
                            out=ps_r[:, :],
                            lhsT=lhs_r_v[:, 0, :],
                            rhs=rhs_v[:, 0, io:io + NT],
                            start=True, stop=False,
                        )
                        nc.tensor.matmul(
                            out=ps_r[:, :],
                            lhsT=lhs_r_v[:, 1, :],
                            rhs=rhs_v[:, 1, io:io + NT],
                            start=False, stop=False,
                        )
                    else:
                        nc.tensor.matmul(
                            out=ps_r[:, :],
                            lhsT=lhs_r_v[:, :, :],
                            rhs=rhs_v[:, :, io:io + NT],
                            start=True, stop=False,
                            perf_mode=mybir.MatmulPerfMode.DoubleRow,
                        )
                    nc.tensor.matmul(
                        out=ps_r[:, :],
                        lhsT=auglr_t[:, :],
                        rhs=augr_sb[:, io:io + NT],
                        start=False, stop=True,
                    )
                    nc.scalar.activation(
                        out=pin[:, io:io + NT], in_=ps_r[:, :],
                        func=mybir.ActivationFunctionType.Copy,
                    )
                if rjt < 12:
                    from concourse.dve_ops import TENSOR_MASK_REDUCE
                    scr_r = scrp.tile([128, SHARD], fp16, tag="scrh")
                    nc.vector._custom_dve(
                        TENSOR_MASK_REDUCE,
                        out=scr_r[:, :],
                        accum_out=realo[:, rjt:rjt + 1],
                        in0=pin[:, :],
                        in1=men_sb[:, rjt:rjt + 1],
                        s0=mst_sb[:, rjt:rjt + 1],
                        s1=FMIN, imm2=1.0,
                    )
                else:
                    nc.vector.tensor_reduce(
                        out=realo[:, rjt:rjt + 1], in_=pin[:, :],
                        axis=mybir.AxisListType.X,
                        op=mybir.AluOpType.max)
                if 12 <= rjt < 12 + NPAR and not NO_PAR:
                    pout = parop.tile([128, SHARD], f32, tag="pout")
                    nc.gpsimd.partition_all_reduce(
                        pout[:, :], pin[:, :], channels=128,
                        reduce_op=bass_isa.ReduceOp.max)
                    nc.sync.dma_start(
                        out=o_par[rjt - 12:rjt - 11, :],
                        in_=pout[0:1, :])

            pending = None
            RSLOTS = 90   # real tiles packed into the first 90 slots so the
            for jt in range(NJT):  # PAR tail drains before the last packs
                jo = jt * JTILE
                do_real = (jt < RSLOTS and
                           (jt * RJT) // RSLOTS != ((jt + 1) * RJT) // RSLOTS)
                rjt = (jt * RJT) // RSLOTS if jt < RSLOTS else 0
                jor = rjt * JTILE
                lhs_g = lhsp.tile([128, 2 * JTILE], gdt, tag="lhs_g")
                lhs_g_v = lhs_g[:, :].rearrange("p (s m) -> p s m", s=2)
                nc.sync.dma_start(out=lhs_g_v[:, :, :],
                                  in_=colg[:, :, jo:jo + JTILE])
                if jt == 0:
                    for it0 in range(1, NIT):
                        io0 = it0 * NT
                        nc.sync.dma_start(out=rhs_v[:, :, io0:io0 + NT],
                                          in_=rhs[:, :, io0:io0 + NT])
                        nc.sync.dma_start(out=augr_sb[:, io0:io0 + NT],
                                          in_=augr[:, io0:io0 + NT])
                if do_real:
                    lhs_r = lhsp.tile([128, 2 * JTILE], gdt, tag="lhs_r")
                    lhs_r_v = lhs_r[:, :].rearrange("p (s m) -> p s m", s=2)
                    nc.sync.dma_start(out=lhs_r_v[:, :, :],
                                      in_=colr[:, :, jor:jor + JTILE])
                    auglr_t = lhsp.tile([128, JTILE], bf16, tag="auglr_t")
                    nc.sync.dma_start(out=auglr_t[:, :],
                                      in_=auglr[:, jor:jor + JTILE])

                # --- gen tile first: one DR matmul per bank + packed scan
                ps_g = psgp.tile([128, SHARD], f32, tag="psg")
                for it in range(NIT):
                    io = it * NT
                    if NO_DR:
                        nc.tensor.matmul(
                            out=ps_g[:, io:io + NT],
                            lhsT=lhs_g_v[:, 0, :],
                            rhs=rhs_v[:, 0, io:io + NT],
                            start=True, stop=False,
                        )
                        nc.tensor.matmul(
                            out=ps_g[:, io:io + NT],
                            lhsT=lhs_g_v[:, 1, :],
                            rhs=rhs_v[:, 1, io:io + NT],
                            start=False, stop=True,
                        )
                    else:
                        nc.tensor.matmul(
                            out=ps_g[:, io:io + NT],
                            lhsT=lhs_g_v[:, :, :],
                            rhs=rhs_v[:, :, io:io + NT],
                            start=True, stop=True,
                            perf_mode=mybir.MatmulPerfMode.DoubleRow,
                        )
                scr_g = scrp.tile([128, SHARD], f32, tag="scr")
                nc.vector._custom_dve(
                    PACK_BIAS, out=scr_g[:, :],
                    accum_out=geno[:, jt:jt + 1],
                    in0=ps_g[:, :], in1=payf_sb[:, :],
                    s0=M_ROUND, s1=0.0, imm2=PSCALE,
                )

                # --- real tile of the PREVIOUS slot (software pipelining:
                # keeps the next gen pack ahead of real work in the queues)
                if pending is not None:
                    emit_real(*pending)
                    pending = None
                if do_real:
                    pending = (rjt, lhs_r_v, auglr_t)
            if pending is not None:
                emit_real(*pending)

            nc.sync.dma_start(out=o_gen[:, :], in_=geno[:, :])
            nc.sync.dma_start(out=o_real[:, :], in_=realo[:, :])

    nc.compile()
    return nc


def _hilo(x, dt):
    hi = x.astype(dt)
    lo = (x - hi.astype(np.float32)).astype(dt)
    return hi, lo


def _pack_dr(a):
    """[256, X] -> DoubleRow layout [128, 2, X] with k = s*128 + p."""
    import ml_dtypes
    dt = ml_dtypes.bfloat16 if NO_DR else ml_dtypes.float8_e4m3fn
    return np.ascontiguousarray(
        a.reshape(2, 128, a.shape[1]).transpose(1, 0, 2)).astype(dt)


def kernel(real_stats, gen_stats, _trace=False):
    import ml_dtypes
    from concourse.bass_utils import run_bass_kernel_spmd

    bf = ml_dtypes.bfloat16
    global _cached_nc
    real = np.ascontiguousarray(np.asarray(real_stats, dtype=np.float32))
    gen = np.ascontiguousarray(np.asarray(gen_stats, dtype=np.float32))

    realT = np.ascontiguousarray(real.T)                  # [D, N]
    genT = np.ascontiguousarray(gen.T)
    colg_f8 = _pack_dr(genT)                              # [128, 2, N]
    b2 = np.sum(real.astype(np.float64) ** 2, axis=1).astype(np.float32)
    a2g = np.sum(gen.astype(np.float64) ** 2, axis=1)

    RW = RJT * JTILE
    iota = np.arange(SHARD, dtype=np.float32)
    p_ar = np.arange(128, dtype=np.float32)
    in_maps = []
    for c in range(NCORES):
        sl = slice(c * SHARD, (c + 1) * SHARD)
        negb2_hi, negb2_lo = _hilo(-8.0 * b2[sl], bf)
        augr_np = np.zeros((128, SHARD), dtype=bf)
        augr_np[0] = negb2_hi
        augr_np[1] = negb2_lo
        augr_np[2:4] = 1
        colr_rot = np.roll(realT, -c * SHARD, axis=1)[:, :RW]
        b2rot = np.roll(b2, -c * SHARD)[:RW]
        nega2_hi, nega2_lo = _hilo(-8.0 * b2rot, bf)
        auglr_np = np.zeros((JTILE, RW), dtype=bf)
        auglr_np[0:2] = 1
        auglr_np[2] = nega2_hi
        auglr_np[3] = nega2_lo
        payf_np = np.tile(
            (np.rint(-8.0 * b2[sl].astype(np.float64)).astype(np.float32)
             * np.float32(PSCALE) + iota), (128, 1))
        # self-tile diagonal masks: exclude free position jor+p via the
        # wrap-around window [d+1, d)
        dpos = p_ar[:, None] + (np.arange(12, dtype=np.float32)
                                * JTILE)[None, :]
        mstart_np = np.ascontiguousarray(dpos + 1.0)
        mend_np = np.ascontiguousarray(dpos)
        in_maps.append({
            "colr": _pack_dr(colr_rot),
            "colg": colg_f8,
            "auglr": auglr_np,
            "rhs": _pack_dr(16.0 * realT[:, sl]),
            "augr": augr_np,
            "payf": np.ascontiguousarray(payf_np),
            "mstart": mstart_np,
            "mend": mend_np,
        })

    if _cached_nc is None:
        _cached_nc = _build_nc()
    res = run_bass_kernel_spmd(_cached_nc, in_maps,
                               core_ids=list(range(NCORES)),
                               trace=_trace)

    # ---- host combine (f64) ----
    cand = np.full(N, np.inf, dtype=np.float64)
    p_idx = np.arange(128)
    for c in range(NCORES):
        rv = res.results[c]["o_real"].astype(np.float64)  # [128, RJT]
        d2 = -rv / 8.0
        jglob = (c * SHARD + np.arange(RJT)[None, :] * JTILE
                 + p_idx[:, None]) % N
        np.minimum.at(cand, jglob.ravel(), d2.ravel())
        par = res.results[c]["o_par"].astype(np.float64)  # [NPAR, SHARD]
        par_d2 = -par.max(axis=0) / 8.0
        sl = slice(c * SHARD, (c + 1) * SHARD)
        cand[sl] = np.minimum(cand[sl], par_d2)
    realNN = np.sqrt(np.maximum(cand, 0.0))               # [N]

    j = np.arange(N)
    P = np.stack([res.results[c]["o_gen"].astype(np.float64)
                  for c in range(NCORES)])                # [8, 128, NJT]
    P = P.transpose(0, 2, 1).reshape(NCORES, N)           # j = jt*128+p
    q = np.floor(P / PSCALE)
    idx = (P - q * PSCALE).astype(np.int64)
    cstar = q.argmax(axis=0)
    d1 = np.sqrt(np.maximum(a2g - q[cstar, j] / 8.0, 0.0))
    istar = cstar * SHARD + idx[cstar, j]
    d2v = realNN[istar]

    z = (d2v - d1) / 0.1
    authen = np.where(z >= 0, 1.0 / (1.0 + np.exp(-np.abs(z))),
                      np.exp(-np.abs(z)) / (1.0 + np.exp(-np.abs(z))))
    out = np.asarray(-100.0 * np.mean(authen), dtype=np.float32)
    if _trace:
        return out, res
    return out


# revision 24
# speedup vs baseline: 1.1726x; 1.1726x over previous
"""AuthPct metric kernel for 8 Trainium2 NeuronCores.

Per core c: rows i = real shard c (1536, moving operand), columns j in
128-wide tiles.  The Gram term 16*f_j.r_i is computed by ONE fp8-e4m3
DoubleRow matmul per 512-i PSUM bank (K=256 packed [128,2,*]); real
tiles add the hi/lo-bf16 norm aug matmul (-8|r_i|^2 - 8|r_j|^2) like the
original baseline, so real PSUM holds complete -8*d^2.

gen side (96 tiles): a single custom DVE op (PACK_BIAS, registered into
dve_ops.OPS at runtime) scans each 3-bank [128,1536] f32 PSUM tile:

    q = round(Src0); P = q*2048 + Src1; accum_out = max_i(P)
    (Src1 payload = round(-8|r_i|^2)*2048 + i)

one 1x pass yields the quantized column max (d^2 to 1/16) AND its argmin
index in the low 11 bits -- replacing the old max + max_index two-pass.

real side (symmetric, shards c..c+4 rotated): every real tile is
evacuated per-bank by ACT to an fp16 SBUF copy; the j-side minima for
ALL 60 tiles (coverage sources t-4..t) use fp16 tensor_mask_reduce at
DVE 2x (the m=0 self tile masks out its diagonal via the wrap-around
start=d+1,end=d window); the i-side minima use Pool partition_all_reduce
on only the m=1..3 copies (sources t+1..t+3) -- 36 PARs instead of the
baseline's 48, since free-side coverage grew to 5 shards.

Host combine: decode q=floor(P/2048), idx=P mod 2048 for gen; real/PAR
values are plain fp16-rounded maxima of -8*d^2; min-combine across
cores, d2 = realNN[argmin], sigmoid, mean.

Measured (HW trace, 306us span): DVE 267us busy (96 packs ~1.86us, 48
tensor_reduce ~1.74us, 12 custom tmr), Pool 36 PAR x 5.4us = 195us, PE
635 matmuls 193us, ACT 180 bank copies 123us, DMA 166us -- DVE-bound.
"""

import os
import numpy as np

NO_DR = bool(int(os.environ.get("V5_NO_DR", "0")))
NO_TMR = bool(int(os.environ.get("V5_NO_TMR", "0")))
NO_PAR = bool(int(os.environ.get("V5_NO_PAR", "0")))

N = 12288
D = 256
NCORES = 8
SHARD = N // NCORES          # 1536 rows per core
JTILE = 128                  # j columns per tile (PSUM partitions)
NJT = N // JTILE             # 96 gen j-tiles
RJT = 60                     # real j-tiles: shards c..c+4 (rotated)
NPAR = 36                    # real j-tiles with PAR harvest (m=1..3)
NT = 512                     # i elements per matmul (PSUM bank)
NIT = SHARD // NT            # 3 i-tiles

M_ROUND = 12582912.0         # 1.5*2^23
PSCALE = 2048.0
FMIN = -3.4028234663852886e38

_cached_nc = None
_pack_ops = None


def _register_pack_ops():
    """Register the PACK_BIAS custom DVE op (idempotent)."""
    global _pack_ops
    if _pack_ops is not None:
        return _pack_ops
    import concourse.dve_ops as dve_ops
    from concourse.dve_spec import (
        Spec, Src0, Src1, C0, C1, C2, MaxNeg, maxx, select, lower,
    )
    from concourse.dve_uop import DveOpSpec
    from concourse.dve_ops import has_src1

    if "PACK_BIAS_ANT" in dve_ops._SUB_OPCODE_FOR_NAME:
        by_name = {op.name: op for op in dve_ops.OPS}
        _pack_ops = (by_name["PACK_BIAS_ANT"],)
        return _pack_ops

    def ref_bias(in0, in1, c0, c1, c2):
        x = np.asarray(in0, np.float32)
        c0 = np.float32(c0) if not isinstance(c0, np.ndarray) else c0.astype(np.float32)
        q = (np.float32(x + c0) - c0).astype(np.float32)
        P = (q * np.float32(c2) + np.asarray(in1, np.float32)).astype(np.float32)
        return P, P.max(axis=-1)

    q = (Src0 + C0) - C0
    P = q * C2 + Src1
    spec_bias = Spec(body=P, accum=maxx, reference=ref_bias)

    ops = []
    for name, spec in (("PACK_BIAS_ANT", spec_bias),):
        row = dve_ops._CUSTOM_DVE_ROW_BASE + len(dve_ops.OPS)
        dve_ops._SUB_OPCODE_FOR_NAME[name] = row
        shas = {}
        for ver in ("v3", "v4"):
            tmp = DveOpSpec(name=name, opcode=row, uops=lower(spec, ver=ver),
                            rd1_en=has_src1(spec))
            shas[ver] = tmp.sha(ver)
        op = dve_ops.DveOp(name, spec, subdim=False, uops_sha=shas)
        dve_ops.OPS.append(op)
        dve_ops.CUSTOM_DVE_SPECS[name] = spec
        ops.append(op)
    assert max(dve_ops._SUB_OPCODE_FOR_NAME.values()) < 0x20
    _pack_ops = tuple(ops)
    return _pack_ops


def _build_nc():
    import concourse.bass_isa as bass_isa
    import concourse.mybir as mybir
    from concourse import bacc
    from concourse.tile import TileContext

    (PACK_BIAS,) = _register_pack_ops()

    f32 = mybir.dt.float32
    fp16 = mybir.dt.float16
    bf16 = mybir.dt.bfloat16
    fp8 = mybir.dt.float8e4

    nc = bacc.Bacc("TRN2", target_bir_lowering=False, debug=False,
                   num_devices=NCORES)

    RW = RJT * JTILE
    # fp8 lhs/rhs packed for DoubleRow: [p, s, x] with k = s*128+p
    gdt = bf16 if NO_DR else fp8
    colr = nc.dram_tensor("colr", [128, 2, RW], gdt, kind="ExternalInput")
    colg = nc.dram_tensor("colg", [128, 2, N], gdt, kind="ExternalInput")
    rhs = nc.dram_tensor("rhs", [128, 2, SHARD], gdt, kind="ExternalInput")
    auglr = nc.dram_tensor("auglr", [JTILE, RW], bf16, kind="ExternalInput")
    augr = nc.dram_tensor("augr", [128, SHARD], bf16, kind="ExternalInput")
    payf = nc.dram_tensor("payf", [128, SHARD], f32, kind="ExternalInput")
    mstart = nc.dram_tensor("mstart", [128, 12], f32, kind="ExternalInput")
    mend = nc.dram_tensor("mend", [128, 12], f32, kind="ExternalInput")

    o_gen = nc.dram_tensor("o_gen", [128, NJT], f32, kind="ExternalOutput")
    o_real = nc.dram_tensor("o_real", [128, RJT], f32, kind="ExternalOutput")
    o_par = nc.dram_tensor("o_par", [NPAR, SHARD], f32, kind="ExternalOutput")

    with TileContext(nc) as tc:
        with (
            tc.tile_pool(name="const", bufs=1) as constp,
            tc.tile_pool(name="lhs", bufs=10) as lhsp,
            tc.tile_pool(name="scr", bufs=4) as scrp,
            tc.tile_pool(name="pari", bufs=8) as parip,
            tc.tile_pool(name="paro", bufs=4) as parop,
            tc.tile_pool(name="outb", bufs=1) as outp,
            tc.tile_pool(name="psg", bufs=2, space="PSUM") as psgp,
            tc.tile_pool(name="psr", bufs=2, space="PSUM") as psrp,
        ):
            # Payload first: the very first gen pack blocks on it.
            payf_sb = constp.tile([128, SHARD], f32)
            nc.sync.dma_start(out=payf_sb[:, :], in_=payf[:, :])
            # Resident moving operand (fp8 DR layout) + aug rows.
            rhs_sb = constp.tile([128, 2 * SHARD], gdt)
            rhs_v = rhs_sb[:, :].rearrange("p (s n) -> p s n", s=2)
            nc.sync.dma_start(out=rhs_v[:, :, 0:NT], in_=rhs[:, :, 0:NT])
            augr_sb = constp.tile([128, SHARD], bf16)
            nc.sync.dma_start(out=augr_sb[:, 0:NT], in_=augr[:, 0:NT])
            mst_sb = constp.tile([128, 12], f32)
            nc.sync.dma_start(out=mst_sb[:, :], in_=mstart[:, :])
            men_sb = constp.tile([128, 12], f32)
            nc.sync.dma_start(out=men_sb[:, :], in_=mend[:, :])
            fullend_sb = constp.tile([128, 1], f32)
            nc.vector.memset(fullend_sb[:, :], float(SHARD))

            geno = outp.tile([128, NJT], f32)
            realo = outp.tile([128, RJT], f32)

            def emit_real(rjt, lhs_r_v, auglr_t):
                pin = parip.tile([128, SHARD], fp16, tag="pin")
                for it in range(NIT):
                    io = it * NT
                    ps_r = psrp.tile([128, NT], f32, tag="psr")
                    if NO_DR:
                        nc.tensor.matmul(
                            out=ps_r[:, :],
                            lhsT=lhs_r_v[:, 0, :],
                            rhs=rhs_v[:, 0, io:io + NT],
                            start=True, stop=False,
                        )
                        nc.tensor.matmul(
                            out=ps_r[:, :],
                            lhsT=lhs_r_v[:, 1, :],
                            rhs=rhs_v[:, 1, io:io + NT],
                            start=False, stop=False,
                        )
                    else:
                        nc.tensor.matmul(
                            out=ps_r[:, :],
                            lhsT=lhs_r_v[:, :, :],
                            rhs=rhs_v[:, :, io:io + NT],
                            start=True, stop=False,
                            perf_mode=mybir.MatmulPerfMode.DoubleRow,
                        )
                    nc.tensor.matmul(
                        out=ps_r[:, :],
                        lhsT=auglr_t[:, :],
                        rhs=augr_sb[:, io:io + NT],
                        start=False, stop=True,
                    )
                    nc.scalar.activation(
                        out=pin[:, io:io + NT], in_=ps_r[:, :],
                        func=mybir.ActivationFunctionType.Copy,
                    )
                if rjt < 12:
                    from concourse.dve_ops import TENSOR_MASK_REDUCE
                    scr_r = scrp.tile([128, SHARD], fp16, tag="scrh")
                    nc.vector._custom_dve(
                        TENSOR_MASK_REDUCE,
                        out=scr_r[:, :],
                        accum_out=realo[:, rjt:rjt + 1],
                        in0=pin[:, :],
                        in1=men_sb[:, rjt:rjt + 1],
                        s0=mst_sb[:, rjt:rjt + 1],
                        s1=FMIN, imm2=1.0,
                    )
                else:
                    nc.vector.tensor_reduce(
                        out=realo[:, rjt:rjt + 1], in_=pin[:, :],
                        axis=mybir.AxisListType.X,
                        op=mybir.AluOpType.max)
                if 12 <= rjt < 12 + NPAR and not NO_PAR:
                    pout = parop.tile([128, SHARD], f32, tag="pout")
                    nc.gpsimd.partition_all_reduce(
                        pout[:, :], pin[:, :], channels=128,
                        reduce_op=bass_isa.ReduceOp.max)
                    nc.sync.dma_start(
                        out=o_par[rjt - 12:rjt - 11, :],
                        in_=pout[0:1, :])

            pending = None
            for jt in range(NJT):
                jo = jt * JTILE
                do_real = (jt * RJT) // NJT != ((jt + 1) * RJT) // NJT
                rjt = (jt * RJT) // NJT
                jor = rjt * JTILE
                lhs_g = lhsp.tile([128, 2 * JTILE], gdt, tag="lhs_g")
                lhs_g_v = lhs_g[:, :].rearrange("p (s m) -> p s m", s=2)
                nc.sync.dma_start(out=lhs_g_v[:, :, :],
                                  in_=colg[:, :, jo:jo + JTILE])
                if jt == 0:
                    for it0 in range(1, NIT):
                        io0 = it0 * NT
                        nc.sync.dma_start(out=rhs_v[:, :, io0:io0 + NT],
                                          in_=rhs[:, :, io0:io0 + NT])
                        nc.sync.dma_start(out=augr_sb[:, io0:io0 + NT],
                                          in_=augr[:, io0:io0 + NT])
                if do_real:
                    lhs_r = lhsp.tile([128, 2 * JTILE], gdt, tag="lhs_r")
                    lhs_r_v = lhs_r[:, :].rearrange("p (s m) -> p s m", s=2)
                    nc.sync.dma_start(out=lhs_r_v[:, :, :],
                                      in_=colr[:, :, jor:jor + JTILE])
                    auglr_t = lhsp.tile([128, JTILE], bf16, tag="auglr_t")
                    nc.sync.dma_start(out=auglr_t[:, :],
                                      in_=auglr[:, jor:jor + JTILE])

                # --- gen tile first: one DR matmul per bank + packed scan
                ps_g = psgp.tile([128, SHARD], f32, tag="psg")
                for it in range(NIT):
                    io = it * NT
                    if NO_DR:
                        nc.tensor.matmul(
                            out=ps_g[:, io:io + NT],
                            lhsT=lhs_g_v[:, 0, :],
                            rhs=rhs_v[:, 0, io:io + NT],
                            start=True, stop=False,
                        )
                        nc.tensor.matmul(
                            out=ps_g[:, io:io + NT],
                            lhsT=lhs_g_v[:, 1, :],
                            rhs=rhs_v[:, 1, io:io + NT],
                            start=False, stop=True,
                        )
                    else:
                        nc.tensor.matmul(
                            out=ps_g[:, io:io + NT],
                            lhsT=lhs_g_v[:, :, :],
                            rhs=rhs_v[:, :, io:io + NT],
                            start=True, stop=True,
                            perf_mode=mybir.MatmulPerfMode.DoubleRow,
                        )
                scr_g = scrp.tile([128, SHARD], f32, tag="scr")
                nc.vector._custom_dve(
                    PACK_BIAS, out=scr_g[:, :],
                    accum_out=geno[:, jt:jt + 1],
                    in0=ps_g[:, :], in1=payf_sb[:, :],
                    s0=M_ROUND, s1=0.0, imm2=PSCALE,
                )

                # --- real tile of the PREVIOUS slot (software pipelining:
                # keeps the next gen pack ahead of real work in the queues)
                if pending is not None:
                    emit_real(*pending)
                    pending = None
                if do_real:
                    pending = (rjt, lhs_r_v, auglr_t)
            if pending is not None:
                emit_real(*pending)

            nc.sync.dma_start(out=o_gen[:, :], in_=geno[:, :])
            nc.sync.dma_start(out=o_real[:, :], in_=realo[:, :])

    nc.compile()
    return nc


def _hilo(x, dt):
    hi = x.astype(dt)
    lo = (x - hi.astype(np.float32)).astype(dt)
    return hi, lo


def _pack_dr(a):
    """[256, X] -> DoubleRow layout [128, 2, X] with k = s*128 + p."""
    import ml_dtypes
    dt = ml_dtypes.bfloat16 if NO_DR else ml_dtypes.float8_e4m3fn
    return np.ascontiguousarray(
        a.reshape(2, 128, a.shape[1]).transpose(1, 0, 2)).astype(dt)


def kernel(real_stats, gen_stats, _trace=False):
    import ml_dtypes
    from concourse.bass_utils import run_bass_kernel_spmd

    bf = ml_dtypes.bfloat16
    global _cached_nc
    real = np.ascontiguousarray(np.asarray(real_stats, dtype=np.float32))
    gen = np.ascontiguousarray(np.asarray(gen_stats, dtype=np.float32))

    realT = np.ascontiguousarray(real.T)                  # [D, N]
    genT = np.ascontiguousarray(gen.T)
    colg_f8 = _pack_dr(genT)                              # [128, 2, N]
    b2 = np.sum(real.astype(np.float64) ** 2, axis=1).astype(np.float32)
    a2g = np.sum(gen.astype(np.float64) ** 2, axis=1)

    RW = RJT * JTILE
    iota = np.arange(SHARD, dtype=np.float32)
    p_ar = np.arange(128, dtype=np.float32)
    in_maps = []
    for c in range(NCORES):
        sl = slice(c * SHARD, (c + 1) * SHARD)
        negb2_hi, negb2_lo = _hilo(-8.0 * b2[sl], bf)
        augr_np = np.zeros((128, SHARD), dtype=bf)
        augr_np[0] = negb2_hi
        augr_np[1] = negb2_lo
        augr_np[2:4] = 1
        colr_rot = np.roll(realT, -c * SHARD, axis=1)[:, :RW]
        b2rot = np.roll(b2, -c * SHARD)[:RW]
        nega2_hi, nega2_lo = _hilo(-8.0 * b2rot, bf)
        auglr_np = np.zeros((JTILE, RW), dtype=bf)
        auglr_np[0:2] = 1
        auglr_np[2] = nega2_hi
        auglr_np[3] = nega2_lo
        payf_np = np.tile(
            (np.rint(-8.0 * b2[sl].astype(np.float64)).astype(np.float32)
             * np.float32(PSCALE) + iota), (128, 1))
        # self-tile diagonal masks: exclude free position jor+p via the
        # wrap-around window [d+1, d)
        dpos = p_ar[:, None] + (np.arange(12, dtype=np.float32)
                                * JTILE)[None, :]
        mstart_np = np.ascontiguousarray(dpos + 1.0)
        mend_np = np.ascontiguousarray(dpos)
        in_maps.append({
            "colr": _pack_dr(colr_rot),
            "colg": colg_f8,
            "auglr": auglr_np,
            "rhs": _pack_dr(16.0 * realT[:, sl]),
            "augr": augr_np,
            "payf": np.ascontiguousarray(payf_np),
            "mstart": mstart_np,
            "mend": mend_np,
        })

    if _cached_nc is None:
        _cached_nc = _build_nc()
    res = run_bass_kernel_spmd(_cached_nc, in_maps,
                               core_ids=list(range(NCORES)),
                               trace=_trace)

    # ---- host combine (f64) ----
    cand = np.full(N, np.inf, dtype=np.float64)
    p_idx = np.arange(128)
    for c in range(NCORES):
        rv = res.results[c]["o_real"].astype(np.float64)  # [128, RJT]
        d2 = -rv / 8.0
        jglob = (c * SHARD + np.arange(RJT)[None, :] * JTILE
                 + p_idx[:, None]) % N
        np.minimum.at(cand, jglob.ravel(), d2.ravel())
        par = res.results[c]["o_par"].astype(np.float64)  # [NPAR, SHARD]
        par_d2 = -par.max(axis=0) / 8.0
        sl = slice(c * SHARD, (c + 1) * SHARD)
        cand[sl] = np.minimum(cand[sl], par_d2)
    realNN = np.sqrt(np.maximum(cand, 0.0))               # [N]

    j = np.arange(N)
    P = np.stack([res.results[c]["o_gen"].astype(np.float64)
                  for c in range(NCORES)])                # [8, 128, NJT]
    P = P.transpose(0, 2, 1).reshape(NCORES, N)           # j = jt*128+p
    q = np.floor(P / PSCALE)
    idx = (P - q * PSCALE).astype(np.int64)
    cstar = q.argmax(axis=0)
    d1 = np.sqrt(np.maximum(a2g - q[cstar, j] / 8.0, 0.0))
    istar = cstar * SHARD + idx[cstar, j]
    d2v = realNN[istar]

    z = (d2v - d1) / 0.1
    authen = np.where(z >= 0, 1.0 / (1.0 + np.exp(-np.abs(z))),
                      np.exp(-np.abs(z)) / (1.0 + np.exp(-np.abs(z))))
    out = np.asarray(-100.0 * np.mean(authen), dtype=np.float32)
    if _trace:
        return out, res
    return out
